# revision 2
# baseline (speedup 1.0000x reference)
"""Trainium2 Bass kernel for nn_Encoder_Decoder: embedding + LSTM over
SEQ=256 steps, BATCH=128, HIDDEN=1024, returning all hidden states.

Strategy (data-parallel, 8 cores, batch 16 per core, no collectives), fully
fused single pipeline:

  The sequence is processed in windows of TCH=16 steps. During window C the
  kernel simultaneously:
    - gathers + PE-transposes the embeddings for window C+2 (X^T in f32r),
    - computes the input projection A = X W_ih^T + b for window C+1
      (W_ih streamed tile-by-tile from DRAM, f32r matmuls, bias folded in
      the psum->SBUF activation, stored bf16 in SBUF — never hits DRAM),
    - runs the LSTM recurrence for the 16 steps of window C.

  Per step, the 4 gates live in 4 separate PSUM banks (order f,g,i,o).
  A[t] is injected into each bank by an identity matmul (start=True), the
  64 W_hh matmuls (bf16, weight-stationary, 16-wide moving dim) accumulate
  on top, so no separate gate add is needed and the activations read PSUM
  directly.  Emission order f->g->i->o lets the c-update chain
  (sig_f, f*c, tanh_g, sig_i, i*g, c, tanh_c) hide under later gate
  matmuls; only sig_o -> h_bf16 sits on the step-boundary critical path.

State layout (everything [128 partitions, .] with col = q*16 + b,
partition p = index within a 128-wide hidden/gate tile):
  h^T, c^T, gate tiles all share the same packing, so the elementwise tail
  needs no data movement and h_bf16 is directly the rhs of the next step's
  W_hh matmuls.
"""

import sys

for _p in ("/opt/trn_rl_repo/concourse", "/opt/trn_rl_repo"):
    if _p not in sys.path:
        sys.path.insert(0, _p)

import numpy as np
import ml_dtypes

SEQ, BATCH, HIDDEN, VOCAB = 256, 128, 1024, 50000
NCORES = 8
CB = BATCH // NCORES          # batch per core = 16
NH = HIDDEN // 128            # hidden k-tiles = 8
NJ = 4 * HIDDEN // 128        # gate j-tiles = 32
TCH = 16                      # steps per window
GW = 8                        # steps per output-staging group

# gate order within a step: block index -> torch gate index (i=0,f=1,g=2,o=3)
BLK2GATE = [1, 2, 0, 3]       # f, g, i, o
GATE2BLK = [2, 0, 1, 3]

_cache = {}


def _build(seq):
    if seq in _cache:
        return _cache[seq]

    import concourse.bass as bass
    import concourse.mybir as mybir
    import concourse.tile as tile
    from concourse import bacc

    f32 = mybir.dt.float32
    f32r = mybir.dt.float32r
    bf16 = mybir.dt.bfloat16
    i32 = mybir.dt.int32
    ACT = mybir.ActivationFunctionType

    W = seq // TCH                # number of windows
    rows_ch = TCH * CB            # rows per window = 256
    nrt_ch = rows_ch // 128       # row-tiles per window = 2
    H4 = 4 * HIDDEN

    nc = bacc.Bacc("TRN2", target_bir_lowering=False, debug=False, num_devices=NCORES)

    # ---------------- DRAM ----------------
    idx_d = nc.dram_tensor("idx", [128, seq * CB // 128], i32, kind="ExternalInput")
    embed_d = nc.dram_tensor("embed", [VOCAB, HIDDEN], f32r, kind="ExternalInput")
    # wih3[p, kb, jj] = W_ih[jj, kb*128+p]          (bf16, contiguous per p)
    wih_d = nc.dram_tensor("wih3", [128, NH, H4], bf16, kind="ExternalInput")
    # whh3[p, kb, jj] = W_hh[jj, kb*128+p]          (bf16, contiguous per p)
    whh_d = nc.dram_tensor("whh3", [128, NH, H4], bf16, kind="ExternalInput")
    bias_d = nc.dram_tensor("biasT", [128, NJ], f32, kind="ExternalInput")
    identr_d = nc.dram_tensor("identr", [128, 128], f32r, kind="ExternalInput")
    identb_d = nc.dram_tensor("identb", [128, 128], bf16, kind="ExternalInput")
    out_d = nc.dram_tensor("out", [seq, 128, NH * CB], f32, kind="ExternalOutput")

    # ---------------- SBUF ----------------
    def sb(name, cols, dtype):
        return nc.alloc_sbuf_tensor(name, [128, cols], dtype).ap()

    whh_sb = sb("whh", NH * H4, bf16)            # 64 KB/part
    wih_sb = sb("wih", NH * H4, bf16)            # 64 KB/part (resident)
    xt_sb = sb("xt", 2 * NH * rows_ch, bf16)     # 8 KB/part (2 windows)
    at_sb = sb("at", 2 * TCH * 512, bf16)        # 32 KB/part (2 windows)
    xr_sb = sb("xr", 2 * HIDDEN, f32r)           # 8 KB/part (2 row-tiles)
    acts_sb = sb("acts", 2 * 512, f32)           # sigmoid/tanh outputs
    gates_sb = sb("gates", 2 * 256, f32)         # f/g pre-activations (psum+A)
    tc_sb = sb("tc", 2 * 128, f32)               # tanh(c)
    t1_sb = sb("t1", 128, f32)
    t2_sb = sb("t2", 128, f32)
    ct_sb = sb("ct", 128, f32)                   # c state (fp32)
    ht_sb = sb("ht", 2 * 128, bf16)              # h state (bf16), per-parity
    hst_sb = sb("hst", 2 * GW * 128, f32)        # output staging
    idx_sb = sb("idx_sb", seq * CB // 128, i32)
    bias_sb = sb("bias_sb", NJ, f32)
    identr_sb = sb("identr_sb", 128, f32r)
    identb_sb = sb("identb_sb", 128, bf16)

    with tile.TileContext(nc) as tc:
        with (
            tc.tile_pool(name="p1ps", bufs=2, space="PSUM") as p1pool,
            tc.tile_pool(name="ptps", bufs=2, space="PSUM") as ptpool,
            tc.tile_pool(name="gateps", bufs=1, space="PSUM") as gatepool,
        ):
            # 4 persistent gate psum tiles (one bank each), reused every step
            pg = [
                gatepool.tile([128, 128], f32, name=f"pg{b}", tag=f"pg{b}")
                for b in range(4)
            ]

            nc.sync.dma_start(idx_sb[:], idx_d[:])
            nc.sync.dma_start(bias_sb[:], bias_d[:])
            nc.sync.dma_start(identr_sb[:], identr_d[:])
            nc.sync.dma_start(identb_sb[:], identb_d[:])
            # whh/wih: 64KB/partition each, 8KB slabs.  wih gates the
            # window-0 projections, so it loads first, spread over the three
            # DMA-capable queues (transfer time serializes per issuing
            # queue); whh follows (not needed until step 1).  Pool first runs
            # the window-0 gathers, then takes two wih slabs.
            for kb in range(NH):
                if kb % 3 != 2:
                    [nc.sync, nc.scalar][kb % 3].dma_start(
                        wih_sb[:, kb * H4 : (kb + 1) * H4], wih_d[:, kb]
                    )
            nc.gpsimd.memset(ht_sb[:], 0.0)
            nc.gpsimd.memset(ct_sb[:], 0.0)

            # ---------------- phase-1 building blocks ----------------
            def gather_rt(ch, rt):
                """indirect-gather row-tile rt (0/1) of window ch into xr."""
                r = ch * nrt_ch + rt
                xr = xr_sb[:, rt * HIDDEN : (rt + 1) * HIDDEN]
                nc.gpsimd.indirect_dma_start(
                    out=xr,
                    out_offset=None,
                    in_=embed_d[:],
                    in_offset=bass.IndirectOffsetOnAxis(
                        ap=idx_sb[:, r : r + 1], axis=0
                    ),
                )

            def transpose_rt(ch, rt, kb):
                """PE-transpose one [128,128] block of xr into xt[ch%2]."""
                xr = xr_sb[:, rt * HIDDEN : (rt + 1) * HIDDEN]
                pt = ptpool.tile([128, 128], f32r, tag="pt")
                nc.tensor.transpose(
                    pt[:], xr[:, kb * 128 : (kb + 1) * 128], identr_sb[:]
                )
                nc.vector.tensor_copy(
                    xt_sb[
                        :,
                        (ch % 2) * NH * rows_ch
                        + kb * rows_ch
                        + rt * 128 : (ch % 2) * NH * rows_ch
                        + kb * rows_ch
                        + (rt + 1) * 128,
                    ],
                    pt[:],
                )

            def jmm(ch, J):
                """input-projection matmul for (window ch, gate tile J) -> at."""
                xtv = xt_sb[
                    :, (ch % 2) * NH * rows_ch : (ch % 2 + 1) * NH * rows_ch
                ]
                pm = p1pool.tile([128, rows_ch], f32, tag="p1")
                for kb in range(NH):
                    nc.tensor.matmul(
                        pm[:],
                        lhsT=wih_sb[:, kb * H4 + J * 128 : kb * H4 + (J + 1) * 128],
                        rhs=xtv[:, kb * rows_ch : (kb + 1) * rows_ch],
                        start=(kb == 0),
                        stop=(kb == NH - 1),
                    )
                gate_t, q = J // NH, J % NH
                col = GATE2BLK[gate_t] * 128 + q * CB
                base = (ch % 2) * TCH * 512
                at_view = at_sb[:, base : base + TCH * 512].rearrange(
                    "p (tt c) -> p tt c", c=512
                )[:, :, col : col + CB]
                nc.scalar.activation(
                    at_view,
                    pm[:].rearrange("p (tt b) -> p tt b", b=CB),
                    ACT.Identity,
                    bias=bias_sb[:, J : J + 1],
                )

            # ---------------- recurrence step ----------------
            def step(t):
                tt = t % TCH
                ch = t // TCH
                a_base = (ch % 2) * TCH * 512 + tt * 512
                apar = t % 2
                acts = acts_sb[:, apar * 512 : (apar + 1) * 512]
                tcv = tc_sb[:, apar * 128 : (apar + 1) * 128]
                ht_in = ht_sb[:, (t % 2) * 128 : (t % 2 + 1) * 128]
                ht_out = ht_sb[:, ((t + 1) % 2) * 128 : ((t + 1) % 2 + 1) * 128]

                def blk_mms(blk, inject):
                    # inject=True: A[t] lands in psum via identity matmul
                    # (start=True); else matmuls start the bank and A is
                    # added on DVE afterwards.
                    gate_t = BLK2GATE[blk]
                    if inject:
                        nc.tensor.matmul(
                            pg[blk][:],
                            lhsT=identb_sb[:],
                            rhs=at_sb[
                                :, a_base + blk * 128 : a_base + (blk + 1) * 128
                            ],
                            start=True,
                            stop=(t == 0),
                            skip_group_check=True,
                        )
                    if t > 0:
                        for q in range(NH):
                            Jt = gate_t * NH + q
                            for kb in range(NH):
                                nc.tensor.matmul(
                                    pg[blk][:, q * CB : (q + 1) * CB],
                                    lhsT=whh_sb[
                                        :,
                                        kb * H4 + Jt * 128 : kb * H4 + (Jt + 1) * 128,
                                    ],
                                    rhs=ht_in[:, kb * CB : (kb + 1) * CB],
                                    start=(not inject and kb == 0),
                                    stop=(kb == NH - 1),
                                    skip_group_check=True,
                                )

                def pre_act(blk, gcol):
                    """pre-activation operand for a DVE-added block."""
                    atv = at_sb[:, a_base + blk * 128 : a_base + (blk + 1) * 128]
                    if t == 0:
                        return atv
                    g = gates_sb[:, apar * 256 + gcol : apar * 256 + gcol + 128]
                    nc.vector.tensor_add(g, pg[blk][:], atv)
                    return g

                # f block, then the f-dependent elementwise
                blk_mms(0, inject=False)
                gf = pre_act(0, 0)
                nc.scalar.activation(acts[:, 0:128], gf, ACT.Sigmoid)
                nc.vector.tensor_mul(t2_sb[:], acts[:, 0:128], ct_sb[:])
                # g block
                blk_mms(1, inject=False)
                gg = pre_act(1, 128)
                nc.scalar.activation(acts[:, 128:256], gg, ACT.Tanh)
                # i block
                blk_mms(2, inject=True)
                nc.scalar.activation(acts[:, 256:384], pg[2][:], ACT.Sigmoid)
                nc.vector.tensor_mul(t1_sb[:], acts[:, 256:384], acts[:, 128:256])
                nc.vector.tensor_add(ct_sb[:], t1_sb[:], t2_sb[:])
                nc.scalar.activation(tcv, ct_sb[:], ACT.Tanh)
                # o block
                blk_mms(3, inject=True)
                nc.scalar.activation(acts[:, 384:512], pg[3][:], ACT.Sigmoid)
                nc.vector.tensor_mul(ht_out, acts[:, 384:512], tcv)
                hoff = ((t // GW) % 2) * GW * 128 + (t % GW) * 128
                hf = hst_sb[:, hoff : hoff + 128]
                nc.vector.tensor_mul(hf, acts[:, 384:512], tcv)
                if t % GW == GW - 1:
                    hst = hst_sb[
                        :,
                        ((t // GW) % 2) * GW * 128 : ((t // GW) % 2 + 1) * GW * 128,
                    ]
                    nc.sync.dma_start(
                        out_d[t - GW + 1 : t + 1].rearrange("t p c -> p t c"),
                        hst.rearrange("p (tt c) -> p tt c", c=128),
                    )

            # ---------------- prologue ----------------
            for rt in range(nrt_ch):
                gather_rt(0, rt)
            for kb in range(NH):
                if kb % 3 == 2:
                    nc.gpsimd.dma_start(
                        wih_sb[:, kb * H4 : (kb + 1) * H4], wih_d[:, kb]
                    )
            for kb in range(NH):
                [nc.sync, nc.scalar][kb % 2].dma_start(
                    whh_sb[:, kb * H4 : (kb + 1) * H4], whh_d[:, kb]
                )
            for rt in range(nrt_ch):
                for kb in range(NH):
                    transpose_rt(0, rt, kb)
            for J in range(NJ):
                jmm(0, J)
            if W > 1:
                for rt in range(nrt_ch):
                    gather_rt(1, rt)
                for rt in range(nrt_ch):
                    for kb in range(NH):
                        transpose_rt(1, rt, kb)

            # J-projection work list: window c's projections run in window c-1
            jwork = [(c, J) for c in range(1, W) for J in range(NJ)]
            jptr = 0

            # ---------------- main loop ----------------
            for t in range(seq):
                step(t)
                s = t % TCH
                # two input-projection tiles per step
                for k in range(2):
                    if jptr < len(jwork):
                        jmm(*jwork[jptr])
                        jptr += 1
                # gather at window start, transposes spread over the window
                c2 = t // TCH + 2
                if c2 < W:
                    if s < nrt_ch:
                        gather_rt(c2, s)
                    elif s < nrt_ch + nrt_ch * NH // 2:
                        for k in (2 * (s - nrt_ch), 2 * (s - nrt_ch) + 1):
                            transpose_rt(c2, k // NH, k % NH)

    nc.compile()
    _cache[seq] = nc
    return nc


def _prep_inputs(inputs, seq):
    input_lines = np.asarray(inputs["input_lines"])[:seq]
    embed = np.ascontiguousarray(np.asarray(inputs["embed_input"], dtype=np.float32))
    wih = np.asarray(inputs["W_ih"], np.float32)      # [4096, 1024]
    whh = np.asarray(inputs["W_hh"], np.float32)
    # w3[p, kb, jj] = W[jj, kb*128+p]
    def w3(w):
        return np.ascontiguousarray(
            w.reshape(4 * HIDDEN, NH, 128).transpose(2, 1, 0).astype(
                ml_dtypes.bfloat16
            )
        )
    wih3 = w3(wih)
    whh3 = w3(whh)
    bias = np.asarray(inputs["b_ih"], np.float32) + np.asarray(inputs["b_hh"], np.float32)
    biasT = np.ascontiguousarray(bias.reshape(NJ, 128).T)
    identr = np.eye(128, dtype=np.float32)
    identb = np.eye(128, dtype=np.float32).astype(ml_dtypes.bfloat16)

    in_maps = []
    for core in range(NCORES):
        sl = input_lines[:, core * CB : (core + 1) * CB].astype(np.int32)
        idx = np.ascontiguousarray(sl.reshape(seq * CB).reshape(-1, 128).T)
        in_maps.append(
            {
                "idx": idx,
                "embed": embed,
                "wih3": wih3,
                "whh3": whh3,
                "biasT": biasT,
                "identr": identr,
                "identb": identb,
            }
        )
    return in_maps


def _assemble(results, seq):
    outs = []
    for core in range(NCORES):
        o = results[core]["out"]  # [seq, 128, 128]: [t, p, 16q+b]
        o = (
            o.reshape(seq, 128, NH, CB)
            .transpose(0, 3, 2, 1)
            .reshape(seq, CB, HIDDEN)
        )
        outs.append(o)
    return np.ascontiguousarray(np.concatenate(outs, axis=1))


def _run(inputs, seq=SEQ, trace=False):
    from concourse.bass_utils import run_bass_kernel_spmd

    nc = _build(seq)
    in_maps = _prep_inputs(inputs, seq)
    res = run_bass_kernel_spmd(
        nc, in_maps, core_ids=list(range(NCORES)), trace=trace
    )
    out = _assemble(res.results, seq)
    if trace:
        return out, res
    return out


def kernel(input_lines, target_lines, embed_input, W_ih, W_hh, b_ih, b_hh):
    return _run(
        {
            "input_lines": input_lines,
            "embed_input": embed_input,
            "W_ih": W_ih,
            "W_hh": W_hh,
            "b_ih": b_ih,
            "b_hh": b_hh,
        },
        seq=SEQ,
    )


# revision 4
# speedup vs baseline: 1.0093x; 1.0093x over previous
"""Trainium2 Bass kernel for nn_Encoder_Decoder: embedding + LSTM over
SEQ=256 steps, BATCH=128, HIDDEN=1024, returning all hidden states.

Strategy (data-parallel, 8 cores, batch 16 per core, no collectives), fully
fused single pipeline:

  The sequence is processed in windows of TCH=16 steps. During window C the
  kernel simultaneously:
    - gathers + PE-transposes the embeddings for window C+2 (X^T in f32r),
    - computes the input projection A = X W_ih^T + b for window C+1
      (W_ih streamed tile-by-tile from DRAM, f32r matmuls, bias folded in
      the psum->SBUF activation, stored bf16 in SBUF — never hits DRAM),
    - runs the LSTM recurrence for the 16 steps of window C.

  Per step, the 4 gates live in 4 separate PSUM banks (order f,g,i,o).
  A[t] is injected into each bank by an identity matmul (start=True), the
  64 W_hh matmuls (bf16, weight-stationary, 16-wide moving dim) accumulate
  on top, so no separate gate add is needed and the activations read PSUM
  directly.  Emission order f->g->i->o lets the c-update chain
  (sig_f, f*c, tanh_g, sig_i, i*g, c, tanh_c) hide under later gate
  matmuls; only sig_o -> h_bf16 sits on the step-boundary critical path.

State layout (everything [128 partitions, .] with col = q*16 + b,
partition p = index within a 128-wide hidden/gate tile):
  h^T, c^T, gate tiles all share the same packing, so the elementwise tail
  needs no data movement and h_bf16 is directly the rhs of the next step's
  W_hh matmuls.
"""

import sys

for _p in ("/opt/trn_rl_repo/concourse", "/opt/trn_rl_repo"):
    if _p not in sys.path:
        sys.path.insert(0, _p)

import numpy as np
import ml_dtypes

SEQ, BATCH, HIDDEN, VOCAB = 256, 128, 1024, 50000
NCORES = 8
CB = BATCH // NCORES          # batch per core = 16
NH = HIDDEN // 128            # hidden k-tiles = 8
NJ = 4 * HIDDEN // 128        # gate j-tiles = 32
TCH = 16                      # steps per window
GW = 8                        # steps per output-staging group

# gate order within a step: block index -> torch gate index (i=0,f=1,g=2,o=3)
BLK2GATE = [1, 2, 0, 3]       # f, g, i, o
GATE2BLK = [2, 0, 1, 3]

_cache = {}


def _build(seq):
    if seq in _cache:
        return _cache[seq]

    import concourse.bass as bass
    import concourse.mybir as mybir
    import concourse.tile as tile
    from concourse import bacc

    f32 = mybir.dt.float32
    f32r = mybir.dt.float32r
    bf16 = mybir.dt.bfloat16
    i32 = mybir.dt.int32
    ACT = mybir.ActivationFunctionType

    W = seq // TCH                # number of windows
    rows_ch = TCH * CB            # rows per window = 256
    nrt_ch = rows_ch // 128       # row-tiles per window = 2
    H4 = 4 * HIDDEN

    nc = bacc.Bacc("TRN2", target_bir_lowering=False, debug=False, num_devices=NCORES)

    # ---------------- DRAM ----------------
    idx_d = nc.dram_tensor("idx", [128, seq * CB // 128], i32, kind="ExternalInput")
    embed_d = nc.dram_tensor("embed", [VOCAB, HIDDEN], f32r, kind="ExternalInput")
    # wih3[p, kb, jj] = W_ih[jj, kb*128+p]          (bf16, contiguous per p)
    wih_d = nc.dram_tensor("wih3", [128, NH, H4], bf16, kind="ExternalInput")
    # whh3[p, kb, jj] = W_hh[jj, kb*128+p]          (bf16, contiguous per p)
    whh_d = nc.dram_tensor("whh3", [128, NH, H4], bf16, kind="ExternalInput")
    bias_d = nc.dram_tensor("biasT", [128, NJ], f32, kind="ExternalInput")
    identr_d = nc.dram_tensor("identr", [128, 128], f32r, kind="ExternalInput")
    identb_d = nc.dram_tensor("identb", [128, 128], bf16, kind="ExternalInput")
    out_d = nc.dram_tensor("out", [seq, 128, NH * CB], f32, kind="ExternalOutput")

    # ---------------- SBUF ----------------
    def sb(name, cols, dtype):
        return nc.alloc_sbuf_tensor(name, [128, cols], dtype).ap()

    whh_sb = sb("whh", NH * H4, bf16)            # 64 KB/part
    wih_sb = sb("wih", NH * H4, bf16)            # 64 KB/part (resident)
    xt_sb = sb("xt", 2 * NH * rows_ch, bf16)     # 8 KB/part (2 windows)
    at_sb = sb("at", 2 * TCH * 512, bf16)        # 32 KB/part (2 windows)
    xr_sb = sb("xr", 2 * HIDDEN, f32r)           # 8 KB/part (2 row-tiles)
    acts_sb = sb("acts", 2 * 512, f32)           # sigmoid/tanh outputs
    gates_sb = sb("gates", 2 * 384, f32)         # f/g/i pre-activations (psum+A)
    tc_sb = sb("tc", 2 * 128, f32)               # tanh(c)
    t1_sb = sb("t1", 128, f32)
    t2_sb = sb("t2", 128, f32)
    ct_sb = sb("ct", 128, f32)                   # c state (fp32)
    ht_sb = sb("ht", 2 * 128, bf16)              # h state (bf16), per-parity
    hst_sb = sb("hst", 2 * GW * 128, f32)        # output staging
    idx_sb = sb("idx_sb", seq * CB // 128, i32)
    bias_sb = sb("bias_sb", NJ, f32)
    identr_sb = sb("identr_sb", 128, f32r)
    identb_sb = sb("identb_sb", 128, bf16)

    with tile.TileContext(nc) as tc:
        with (
            tc.tile_pool(name="p1ps", bufs=4, space="PSUM") as p1pool,
            tc.tile_pool(name="gateps", bufs=1, space="PSUM") as gatepool,
        ):
            ptpool = p1pool  # transposes share the projection psum slots
            # 4 persistent gate psum tiles (one bank each), reused every step
            pg = [
                gatepool.tile([128, 128], f32, name=f"pg{b}", tag=f"pg{b}")
                for b in range(4)
            ]

            nc.sync.dma_start(idx_sb[:], idx_d[:])
            nc.sync.dma_start(bias_sb[:], bias_d[:])
            nc.sync.dma_start(identr_sb[:], identr_d[:])
            nc.sync.dma_start(identb_sb[:], identb_d[:])
            # whh/wih: 64KB/partition each, 8KB slabs.  wih gates the
            # window-0 projections, so it loads first, spread over the three
            # DMA-capable queues (transfer time serializes per issuing
            # queue); whh follows (not needed until step 1).  Pool first runs
            # the window-0 gathers, then takes two wih slabs.
            for kb in range(NH):
                if kb % 3 != 2:
                    [nc.sync, nc.scalar][kb % 3].dma_start(
                        wih_sb[:, kb * H4 : (kb + 1) * H4], wih_d[:, kb]
                    )
            nc.gpsimd.memset(ht_sb[:], 0.0)
            nc.gpsimd.memset(ct_sb[:], 0.0)

            # ---------------- phase-1 building blocks ----------------
            def gather_rt(ch, rt):
                """indirect-gather row-tile rt (0/1) of window ch into xr."""
                r = ch * nrt_ch + rt
                xr = xr_sb[:, rt * HIDDEN : (rt + 1) * HIDDEN]
                nc.gpsimd.indirect_dma_start(
                    out=xr,
                    out_offset=None,
                    in_=embed_d[:],
                    in_offset=bass.IndirectOffsetOnAxis(
                        ap=idx_sb[:, r : r + 1], axis=0
                    ),
                )

            def transpose_rt(ch, rt, kb):
                """PE-transpose one [128,128] block of xr into xt[ch%2]."""
                xr = xr_sb[:, rt * HIDDEN : (rt + 1) * HIDDEN]
                pt = ptpool.tile([128, 128], f32r, tag="p1")
                nc.tensor.transpose(
                    pt[:], xr[:, kb * 128 : (kb + 1) * 128], identr_sb[:]
                )
                nc.vector.tensor_copy(
                    xt_sb[
                        :,
                        (ch % 2) * NH * rows_ch
                        + kb * rows_ch
                        + rt * 128 : (ch % 2) * NH * rows_ch
                        + kb * rows_ch
                        + (rt + 1) * 128,
                    ],
                    pt[:],
                )

            def jmm_half(ch, J, h):
                """input projection for (window ch, gate tile J), steps
                [8h, 8h+8) of the window -> at.  Half-window quanta let
                window 0 start after only half its projections, and let
                phase-1 filler run 8 steps into the final window."""
                HR = rows_ch // 2
                xb = (ch % 2) * NH * rows_ch + h * HR
                pm = p1pool.tile([128, HR], f32, tag="p1")
                for kb in range(NH):
                    nc.tensor.matmul(
                        pm[:],
                        lhsT=wih_sb[:, kb * H4 + J * 128 : kb * H4 + (J + 1) * 128],
                        rhs=xt_sb[:, xb + kb * rows_ch : xb + kb * rows_ch + HR],
                        start=(kb == 0),
                        stop=(kb == NH - 1),
                    )
                gate_t, q = J // NH, J % NH
                col = GATE2BLK[gate_t] * 128 + q * CB
                base = (ch % 2) * TCH * 512
                at_view = at_sb[:, base : base + TCH * 512].rearrange(
                    "p (tt c) -> p tt c", c=512
                )[:, h * TCH // 2 : (h + 1) * TCH // 2, col : col + CB]
                nc.scalar.activation(
                    at_view,
                    pm[:].rearrange("p (tt b) -> p tt b", b=CB),
                    ACT.Identity,
                    bias=bias_sb[:, J : J + 1],
                )

            # ---------------- recurrence step ----------------
            def step(t):
                tt = t % TCH
                ch = t // TCH
                a_base = (ch % 2) * TCH * 512 + tt * 512
                apar = t % 2
                acts = acts_sb[:, apar * 512 : (apar + 1) * 512]
                tcv = tc_sb[:, apar * 128 : (apar + 1) * 128]
                ht_in = ht_sb[:, (t % 2) * 128 : (t % 2 + 1) * 128]
                ht_out = ht_sb[:, ((t + 1) % 2) * 128 : ((t + 1) % 2 + 1) * 128]

                def blk_mms(blk, inject):
                    # inject=True: A[t] lands in psum via identity matmul
                    # (start=True); else matmuls start the bank and A is
                    # added on DVE afterwards.
                    gate_t = BLK2GATE[blk]
                    if inject:
                        nc.tensor.matmul(
                            pg[blk][:],
                            lhsT=identb_sb[:],
                            rhs=at_sb[
                                :, a_base + blk * 128 : a_base + (blk + 1) * 128
                            ],
                            start=True,
                            stop=(t == 0),
                            skip_group_check=True,
                        )
                    if t > 0:
                        for q in range(NH):
                            Jt = gate_t * NH + q
                            for kb in range(NH):
                                nc.tensor.matmul(
                                    pg[blk][:, q * CB : (q + 1) * CB],
                                    lhsT=whh_sb[
                                        :,
                                        kb * H4 + Jt * 128 : kb * H4 + (Jt + 1) * 128,
                                    ],
                                    rhs=ht_in[:, kb * CB : (kb + 1) * CB],
                                    start=(not inject and kb == 0),
                                    stop=(kb == NH - 1),
                                    skip_group_check=True,
                                )

                def pre_act(blk, gcol):
                    """pre-activation operand for a DVE-added block."""
                    atv = at_sb[:, a_base + blk * 128 : a_base + (blk + 1) * 128]
                    if t == 0:
                        return atv
                    g = gates_sb[:, apar * 384 + gcol : apar * 384 + gcol + 128]
                    nc.vector.tensor_add(g, pg[blk][:], atv)
                    return g

                # f block, then the f-dependent elementwise
                blk_mms(0, inject=False)
                gf = pre_act(0, 0)
                nc.scalar.activation(acts[:, 0:128], gf, ACT.Sigmoid)
                nc.vector.tensor_mul(t2_sb[:], acts[:, 0:128], ct_sb[:])
                # g block
                blk_mms(1, inject=False)
                gg = pre_act(1, 128)
                nc.scalar.activation(acts[:, 128:256], gg, ACT.Tanh)
                # i block
                blk_mms(2, inject=True)
                nc.scalar.activation(acts[:, 256:384], pg[2][:], ACT.Sigmoid)
                nc.vector.tensor_mul(t1_sb[:], acts[:, 256:384], acts[:, 128:256])
                nc.vector.tensor_add(ct_sb[:], t1_sb[:], t2_sb[:])
                nc.scalar.activation(tcv, ct_sb[:], ACT.Tanh)
                # o block
                blk_mms(3, inject=True)
                nc.scalar.activation(acts[:, 384:512], pg[3][:], ACT.Sigmoid)
                nc.vector.tensor_mul(ht_out, acts[:, 384:512], tcv)
                hoff = ((t // GW) % 2) * GW * 128 + (t % GW) * 128
                hf = hst_sb[:, hoff : hoff + 128]
                nc.vector.tensor_mul(hf, acts[:, 384:512], tcv)
                if t % GW == GW - 1:
                    hst = hst_sb[
                        :,
                        ((t // GW) % 2) * GW * 128 : ((t // GW) % 2 + 1) * GW * 128,
                    ]
                    nc.sync.dma_start(
                        out_d[t - GW + 1 : t + 1].rearrange("t p c -> p t c"),
                        hst.rearrange("p (tt c) -> p tt c", c=128),
                    )

            # ---------------- prologue ----------------
            for rt in range(nrt_ch):
                gather_rt(0, rt)
            for kb in range(NH):
                if kb % 3 == 2:
                    nc.gpsimd.dma_start(
                        wih_sb[:, kb * H4 : (kb + 1) * H4], wih_d[:, kb]
                    )
            for kb in range(NH):
                [nc.sync, nc.scalar][kb % 2].dma_start(
                    whh_sb[:, kb * H4 : (kb + 1) * H4], whh_d[:, kb]
                )
            for rt in range(nrt_ch):
                for kb in range(NH):
                    transpose_rt(0, rt, kb)
            # projection quanta in deadline order: quantum (c, J, h) must
            # land before step 16c+8h.  Prologue runs the first 40 (all of
            # window 0's first half + a head start); the loop then paces 4
            # per step, which keeps >=3 steps of slack on every deadline
            # and carries filler 8 steps into the final window.
            qwork = [
                (c, J, h) for c in range(W) for h in range(2) for J in range(NJ)
            ]
            qptr = 0
            while qptr < min(40, len(qwork)):
                jmm_half(*qwork[qptr])
                qptr += 1
            if W > 1:
                for rt in range(nrt_ch):
                    gather_rt(1, rt)
                for rt in range(nrt_ch):
                    for kb in range(NH):
                        transpose_rt(1, rt, kb)

            # ---------------- main loop ----------------
            for t in range(seq):
                step(t)
                s = t % TCH
                for k in range(4):
                    if qptr < len(qwork):
                        jmm_half(*qwork[qptr])
                        qptr += 1
                # gather at window start, transposes spread over the window
                c2 = t // TCH + 2
                if c2 < W:
                    if s < nrt_ch:
                        gather_rt(c2, s)
                    elif s < nrt_ch + nrt_ch * NH // 2:
                        for k in (2 * (s - nrt_ch), 2 * (s - nrt_ch) + 1):
                            transpose_rt(c2, k // NH, k % NH)

    nc.compile()
    _cache[seq] = nc
    return nc


def _prep_inputs(inputs, seq):
    input_lines = np.asarray(inputs["input_lines"])[:seq]
    embed = np.ascontiguousarray(np.asarray(inputs["embed_input"], dtype=np.float32))
    wih = np.asarray(inputs["W_ih"], np.float32)      # [4096, 1024]
    whh = np.asarray(inputs["W_hh"], np.float32)
    # w3[p, kb, jj] = W[jj, kb*128+p]
    def w3(w):
        return np.ascontiguousarray(
            w.reshape(4 * HIDDEN, NH, 128).transpose(2, 1, 0).astype(
                ml_dtypes.bfloat16
            )
        )
    wih3 = w3(wih)
    whh3 = w3(whh)
    bias = np.asarray(inputs["b_ih"], np.float32) + np.asarray(inputs["b_hh"], np.float32)
    biasT = np.ascontiguousarray(bias.reshape(NJ, 128).T)
    identr = np.eye(128, dtype=np.float32)
    identb = np.eye(128, dtype=np.float32).astype(ml_dtypes.bfloat16)

    in_maps = []
    for core in range(NCORES):
        sl = input_lines[:, core * CB : (core + 1) * CB].astype(np.int32)
        idx = np.ascontiguousarray(sl.reshape(seq * CB).reshape(-1, 128).T)
        in_maps.append(
            {
                "idx": idx,
                "embed": embed,
                "wih3": wih3,
                "whh3": whh3,
                "biasT": biasT,
                "identr": identr,
                "identb": identb,
            }
        )
    return in_maps


def _assemble(results, seq):
    outs = []
    for core in range(NCORES):
        o = results[core]["out"]  # [seq, 128, 128]: [t, p, 16q+b]
        o = (
            o.reshape(seq, 128, NH, CB)
            .transpose(0, 3, 2, 1)
            .reshape(seq, CB, HIDDEN)
        )
        outs.append(o)
    return np.ascontiguousarray(np.concatenate(outs, axis=1))


def _run(inputs, seq=SEQ, trace=False):
    from concourse.bass_utils import run_bass_kernel_spmd

    nc = _build(seq)
    in_maps = _prep_inputs(inputs, seq)
    res = run_bass_kernel_spmd(
        nc, in_maps, core_ids=list(range(NCORES)), trace=trace
    )
    out = _assemble(res.results, seq)
    if trace:
        return out, res
    return out


def kernel(input_lines, target_lines, embed_input, W_ih, W_hh, b_ih, b_hh):
    return _run(
        {
            "input_lines": input_lines,
            "embed_input": embed_input,
            "W_ih": W_ih,
            "W_hh": W_hh,
            "b_ih": b_ih,
            "b_hh": b_hh,
        },
        seq=SEQ,
    )
